# revision 6
# baseline (speedup 1.0000x reference)
"""Bahdanau additive attention on 8 TRN2 NeuronCores (batch-parallel).

Math: scores[b,i,j] = q[b,i].w + k[b,j].w, masked to -1e9 where mask==0,
softmax over j, then @ value.  The query term q[b,i].w is constant along j,
so it cancels in the softmax:

    out[b,i,:] = (sum_j mask[b,i,j] * e[b,j] * value[b,j,:])
               / (sum_j mask[b,i,j] * e[b,j]),      e[b,j] = exp(k[b,j].w)

(no query needed, no [Lq,Lk] softmax).  Per core: one batch.

Layout strategy: the PE contracts over partitions, so the mask needs j on
partitions.  Rather than transposing on-chip (256 PE transposes ~ 27us),
the host uploads the mask PRE-TRANSPOSED as uint8 in j-major tile order:
maskt[p, s, t*128+c] = mask[i=128t+c, j=128s+p].  That's 4x fewer HBM
bytes than int32 and removes all PE transpose work.

The softmax weight e_j is folded into the STATIONARY operand: the 0/1 mask
bytes become fp16 m*e_j while casting, via three parallel converters:
  - SWDGE cast-DMA (u8 -> f16 during the DMA, gpsimd ring); these strips
    stay 0/1 and instead use a pre-scaled moving operand [e*v | e],
  - DVE tensor_scalar mult (u8 in, per-partition scalar e_j, f16 out),
  - ACT activation-copy with scale=e_j (u8 in, f16 out).
The moving operand for scaled-mask strips is the raw fp16 value with a
ones-column appended ON THE HOST (col 256 = 1.0), so psum col 256
accumulates the softmax denominator with zero extra device work.
16 i-tiles run in two waves of 8 psum banks; epilogue divides, stores f16.

A dependency-free burst of dummy matmuls at kernel start trips the PE HAM
activity monitor to full clock before real work arrives.
"""

import os
import sys
import types

sys.path.insert(0, "/opt/trn_rl_repo")

import numpy as np

import concourse.bacc as bacc
import concourse.tile as tile
from concourse import mybir
from concourse.bass_utils import run_bass_kernel_spmd


def _ensure_ntff_hook_importable():
    """bass_utils imports antenv.axon_hooks when BASS_TRACE is set; this
    image's antenv lacks that module.  Provide it (and register the real
    ctypes NTFF hook if available) so tracing works instead of crashing."""
    if "antenv.axon_hooks" in sys.modules:
        return
    try:
        import antenv
    except ImportError:
        return
    hooks = types.ModuleType("antenv.axon_hooks")
    hooks._hook = None
    hooks.set_axon_ntff_profile_hook = lambda h: setattr(hooks, "_hook", h)
    hooks.get_axon_ntff_profile_hook = lambda: hooks._hook
    sys.modules["antenv.axon_hooks"] = hooks
    antenv.axon_hooks = hooks
    try:
        from trn_agent_boot.trn_boot import _ntff_profile_via_ctypes

        hook = _ntff_profile_via_ctypes("/opt/axon/libaxon_pjrt.so")
        if hook is not None:
            hooks.set_axon_ntff_profile_hook(hook)
    except Exception:
        pass


_ensure_ntff_hook_importable()

P = 128
B = 8
L = 2048
D = 256
NT = L // P  # 16 tiles per dim
NE = D + 1  # 257 = value cols + denominator col (matmul moving width)
VP = D + 2  # 258 = value/ev row pitch (even, for engine perf modes)

# strip -> converter assignment (tunable)
CAST_STRIPS = (0, 3, 6, 9, 12, 15)  # SWDGE u8->f16 cast-DMA; moving = [e*v|e]
DVE_STRIPS = (1, 4, 7, 10, 13)  # u8 load + DVE mult-by-e cast
ACT_STRIPS = (2, 5, 8, 11, 14)  # u8 load + ACT copy-with-scale cast
N_WARM = 6

LAST_RESULTS = None


def _build_nc():
    dt = mybir.dt
    nc = bacc.Bacc("TRN2", target_bir_lowering=False, debug=False, num_devices=B)

    maskt_d = nc.dram_tensor("maskt", [P, NT * L], dt.uint8, kind="ExternalInput").ap()
    key_d = nc.dram_tensor("key", [P, NT * D], dt.float32, kind="ExternalInput").ap()
    value_d = nc.dram_tensor("value", [P, NT * VP], dt.float16, kind="ExternalInput").ap()
    wrep_d = nc.dram_tensor("wrep", [P, D], dt.float32, kind="ExternalInput").ap()
    out_d = nc.dram_tensor("out", [P, NT * D], dt.float16, kind="ExternalOutput").ap()

    with tile.TileContext(nc) as tc:
        with (
            tc.tile_pool(name="const", bufs=1) as const_pool,
            tc.tile_pool(name="kv", bufs=1) as kv_pool,
            tc.tile_pool(name="small", bufs=1) as small_pool,
            tc.tile_pool(name="junk", bufs=2) as junk_pool,
            tc.tile_pool(name="mu8", bufs=10) as mu8_pool,
            tc.tile_pool(name="outp", bufs=4) as out_pool,
            tc.tile_pool(name="rec", bufs=4) as rec_pool,
            tc.tile_pool(name="acc", bufs=8, space="PSUM") as acc_pool,
        ):
            # HAM warmup: dummy matmuls with no real dependencies (zeroed
            # data; results never read) to bring the PE to full clock.
            warm_mv = const_pool.tile([P, 512], dt.float16)
            nc.vector.memset(warm_mv[:], 0.0)
            warm_ps = acc_pool.tile([P, 512], dt.float32, tag="acc", name="warm")
            for _ in range(N_WARM):
                nc.tensor.matmul(
                    warm_ps[:], warm_mv[:, 0:P], warm_mv[:], start=True, stop=True
                )

            # ---- DMAs: scalar ring carries wrep + k/v chunks; gpsimd ring
            # carries the cast strips; sync ring carries u8 strips (+ output
            # stores later).
            wrep = const_pool.tile([P, D], dt.float32)
            nc.scalar.dma_start(wrep[:], wrep_d[:])
            k_sb = kv_pool.tile([P, NT * D], dt.float32, tag="ksb")
            v_sb = kv_pool.tile([P, NT * VP], dt.float16, tag="vsb")
            for c in range(4):
                ks = slice(c * 4 * D, (c + 1) * 4 * D)
                vs = slice(c * 4 * VP, (c + 1) * 4 * VP)
                nc.scalar.dma_start(k_sb[:, ks], key_d[:, ks])
                nc.scalar.dma_start(v_sb[:, vs], value_d[:, vs])
            v3 = v_sb[:].rearrange("p (s n) -> p s n", n=VP)

            mask16 = kv_pool.tile([P, NT * L], dt.float16, tag="m16")
            m16v = mask16[:].rearrange("p (s i) -> p s i", s=NT)
            mu8 = {}
            for s in range(NT):
                sl = slice(s * L, (s + 1) * L)
                if s in CAST_STRIPS:
                    nc.gpsimd.dma_start(mask16[:, sl], maskt_d[:, sl])
                else:
                    t8 = mu8_pool.tile([P, L], dt.uint8, tag="mu8", name=f"mu8_{s}")
                    nc.sync.dma_start(t8[:], maskt_d[:, sl])
                    mu8[s] = t8

            # ---- prologue per chunk of 4 strips: sk = k.w ; e = exp(sk) ;
            # then scaled-mask conversions / ev rows for cast strips.
            sk = small_pool.tile([P, NT], dt.float32, tag="sk")
            e_sb = small_pool.tile([P, NT], dt.float32, tag="e")
            ev = kv_pool.tile([P, NT * VP], dt.float16, tag="ev")
            ev3 = ev[:].rearrange("p (s n) -> p s n", n=VP)
            for c in range(4):
                for s in range(4 * c, 4 * c + 4):
                    junk = junk_pool.tile([P, D], dt.float32, tag="junk")
                    nc.vector.scalar_tensor_tensor(
                        out=junk[:],
                        in0=k_sb[:, s * D : (s + 1) * D],
                        scalar=1.0,
                        in1=wrep[:],
                        op0=mybir.AluOpType.mult,
                        op1=mybir.AluOpType.mult,
                        accum_out=sk[:, s : s + 1],
                    )
                cs = slice(4 * c, 4 * c + 4)
                nc.scalar.activation(
                    e_sb[:, cs], sk[:, cs], mybir.ActivationFunctionType.Exp
                )
                nc.vector.tensor_copy(ev3[:, cs, D], e_sb[:, cs])
                for s in range(4 * c, 4 * c + 4):
                    if s in CAST_STRIPS:
                        # moving for this strip: ev row [e*v | e | 0]
                        nc.vector.tensor_scalar_mul(
                            ev3[:, s, 0:D], v3[:, s, 0:D], e_sb[:, s : s + 1]
                        )
                    elif s in DVE_STRIPS:
                        nc.vector.tensor_scalar_mul(
                            mask16[:, s * L : (s + 1) * L],
                            mu8[s][:],
                            e_sb[:, s : s + 1],
                        )
                    else:
                        nc.scalar.mul(
                            mask16[:, s * L : (s + 1) * L],
                            mu8[s][:],
                            e_sb[:, s : s + 1],
                        )

            # ---- two waves of 8 i-tiles; 16 accumulating matmuls each
            for w in range(2):
                accs = []
                for t in range(8 * w, 8 * w + 8):
                    accs.append(
                        acc_pool.tile([P, NE], dt.float32, tag="acc", name=f"acc{t}")
                    )
                for s in range(NT):
                    mov = ev3 if s in CAST_STRIPS else v3
                    for ti, t in enumerate(range(8 * w, 8 * w + 8)):
                        nc.tensor.matmul(
                            accs[ti][:],
                            m16v[:, s, t * P : (t + 1) * P],
                            mov[:, s, 0:NE],
                            start=(s == 0),
                            stop=(s == NT - 1),
                        )
                for ti, t in enumerate(range(8 * w, 8 * w + 8)):
                    acc = accs[ti]
                    rec = rec_pool.tile([P, 1], dt.float32, tag="rec")
                    nc.vector.reciprocal(rec[:], acc[:, D : D + 1])
                    outt = out_pool.tile([P, D], dt.float16, tag="outt")
                    nc.scalar.mul(outt[:], acc[:, 0:D], rec[:])
                    nc.sync.dma_start(out_d[:, t * D : (t + 1) * D], outt[:])

    nc.compile()
    return nc


def kernel(query, key, value, mask, w_align):
    global LAST_RESULTS
    key = np.asarray(key, dtype=np.float32)
    value = np.asarray(value, dtype=np.float32)
    mask = np.asarray(mask)
    w_align = np.asarray(w_align, dtype=np.float32)
    wrep = np.ascontiguousarray(np.tile(w_align[None, :], (P, 1)))

    nc = _build_nc()
    in_maps = []
    for b in range(B):
        # maskt[p, s, t*128+c] = mask[b][i=128t+c, j=128s+p]
        mt = (
            mask[b]
            .astype(np.uint8)
            .reshape(NT, P, NT, P)  # [t, c, s, p]
            .transpose(3, 2, 0, 1)  # [p, s, t, c]
            .reshape(P, NT * L)
        )
        kb = np.ascontiguousarray(
            key[b].reshape(NT, P, D).transpose(1, 0, 2).reshape(P, NT * D)
        )
        # value with ones-column (denominator) and zero pad, pitch 258
        vb = np.zeros((P, NT, VP), dtype=np.float16)
        vb[:, :, 0:D] = value[b].reshape(NT, P, D).transpose(1, 0, 2)
        vb[:, :, D] = 1.0
        in_maps.append(
            {
                "maskt": np.ascontiguousarray(mt),
                "key": kb,
                "value": vb.reshape(P, NT * VP),
                "wrep": wrep,
            }
        )
    try:
        res = run_bass_kernel_spmd(nc, in_maps, core_ids=list(range(B)))
    except Exception:
        # e.g. trace requested but profiling unavailable -- retry untraced
        os.environ["BASS_NEVER_TRACE"] = "1"
        res = run_bass_kernel_spmd(nc, in_maps, core_ids=list(range(B)))
    LAST_RESULTS = res
    out = np.empty((B, L, D), dtype=np.float32)
    for b in range(B):
        ob = res.results[b]["out"].astype(np.float32)  # [p, t*D]
        out[b] = ob.reshape(P, NT, D).transpose(1, 0, 2).reshape(L, D)
    return out


# revision 7
# speedup vs baseline: 1.1849x; 1.1849x over previous
"""Bahdanau additive attention on 8 TRN2 NeuronCores (batch-parallel).

Math: scores[b,i,j] = q[b,i].w + k[b,j].w, masked to -1e9 where mask==0,
softmax over j, then @ value.  The query term q[b,i].w is constant along j,
so it cancels in the softmax:

    out[b,i,:] = (sum_j mask[b,i,j] * e[b,j] * value[b,j,:])
               / (sum_j mask[b,i,j] * e[b,j]),      e[b,j] = exp(k[b,j].w)

(no query needed, no [Lq,Lk] softmax).  Per core: one batch.

Layout strategy: the PE contracts over partitions, so the mask needs j on
partitions.  Rather than transposing on-chip (256 PE transposes ~ 27us),
the host uploads the mask PRE-TRANSPOSED as uint8 in j-major tile order:
maskt[p, s, t*128+c] = mask[i=128t+c, j=128s+p].  That's 4x fewer HBM
bytes than int32 and removes all PE transpose work.

The softmax weight e_j is folded into the STATIONARY operand: the 0/1 mask
bytes become fp16 m*e_j while casting, via three parallel converters:
  - SWDGE cast-DMA (u8 -> f16 during the DMA, gpsimd ring); these strips
    stay 0/1 and instead use a pre-scaled moving operand [e*v | e],
  - DVE tensor_scalar mult (u8 in, per-partition scalar e_j, f16 out),
  - ACT activation-copy with scale=e_j (u8 in, f16 out).
The moving operand for scaled-mask strips is the raw fp16 value with a
ones-column appended ON THE HOST (col 256 = 1.0), so psum col 256
accumulates the softmax denominator with zero extra device work.
16 i-tiles run in two waves of 8 psum banks; epilogue divides, stores f16.

DMA ordering: SDMA engines round-robin across rings at packet granularity,
so a transfer's completion time is set by the total backlog, not its own
size.  But WITHIN one HWDGE ring, transfers complete FIFO.  Everything
order-critical (wrep -> k0 -> v0 -> u8 strip pairs interleaved with later
k/v chunks) goes on the sync ring in consumption order; cast strips ride
the independent SWDGE ring; the two coalesced output stores are the only
scalar-ring DMAs so the ACT queue is never blocked behind a dispatch.

A dependency-free burst of dummy matmuls at kernel start trips the PE HAM
activity monitor to full clock before real work arrives.
"""

import os
import sys
import types

sys.path.insert(0, "/opt/trn_rl_repo")

import numpy as np

import concourse.bacc as bacc
import concourse.tile as tile
from concourse import mybir
from concourse.bass_utils import run_bass_kernel_spmd


def _ensure_ntff_hook_importable():
    """bass_utils imports antenv.axon_hooks when BASS_TRACE is set; this
    image's antenv lacks that module.  Provide it (and register the real
    ctypes NTFF hook if available) so tracing works instead of crashing."""
    if "antenv.axon_hooks" in sys.modules:
        return
    try:
        import antenv
    except ImportError:
        return
    hooks = types.ModuleType("antenv.axon_hooks")
    hooks._hook = None
    hooks.set_axon_ntff_profile_hook = lambda h: setattr(hooks, "_hook", h)
    hooks.get_axon_ntff_profile_hook = lambda: hooks._hook
    sys.modules["antenv.axon_hooks"] = hooks
    antenv.axon_hooks = hooks
    try:
        from trn_agent_boot.trn_boot import _ntff_profile_via_ctypes

        hook = _ntff_profile_via_ctypes("/opt/axon/libaxon_pjrt.so")
        if hook is not None:
            hooks.set_axon_ntff_profile_hook(hook)
    except Exception:
        pass


_ensure_ntff_hook_importable()

P = 128
B = 8
L = 2048
D = 256
NT = L // P  # 16 tiles per dim
NE = D + 1  # 257 = value cols + denominator col (matmul moving width)
VP = D + 2  # 258 = value/ev row pitch (even, for engine perf modes)

# strip -> converter assignment (tunable)
CAST_STRIPS = (0, 3, 6, 9, 12, 15)  # SWDGE u8->f16 cast-DMA; moving = [e*v|e]
DVE_STRIPS = (1, 4, 7, 10, 13)  # u8 load + DVE mult-by-e cast
ACT_STRIPS = (2, 5, 8, 11, 14)  # u8 load + ACT copy-with-scale cast
U8_PAIRS = ((1, 2), (4, 5), (7, 8), (10, 11), (13, 14))
N_WARM = 9

LAST_RESULTS = None


def _build_nc():
    dt = mybir.dt
    nc = bacc.Bacc("TRN2", target_bir_lowering=False, debug=False, num_devices=B)

    maskt_d = nc.dram_tensor("maskt", [P, NT * L], dt.uint8, kind="ExternalInput").ap()
    key_d = nc.dram_tensor("key", [P, NT * D], dt.float32, kind="ExternalInput").ap()
    value_d = nc.dram_tensor("value", [P, NT * VP], dt.float16, kind="ExternalInput").ap()
    wrep_d = nc.dram_tensor("wrep", [P, D], dt.float32, kind="ExternalInput").ap()
    out_d = nc.dram_tensor("out", [P, NT * D], dt.float16, kind="ExternalOutput").ap()

    with tile.TileContext(nc) as tc:
        with (
            tc.tile_pool(name="const", bufs=1) as const_pool,
            tc.tile_pool(name="kv", bufs=1) as kv_pool,
            tc.tile_pool(name="small", bufs=1) as small_pool,
            tc.tile_pool(name="junk", bufs=2) as junk_pool,
            tc.tile_pool(name="mu8", bufs=5) as mu8_pool,
            tc.tile_pool(name="outp", bufs=2) as out_pool,
            tc.tile_pool(name="rec", bufs=4) as rec_pool,
            tc.tile_pool(name="acc", bufs=8, space="PSUM") as acc_pool,
        ):
            # HAM warmup: dummy matmuls with no real dependencies (zeroed
            # data; results never read) to bring the PE to full clock.
            # memset on gpsimd: the vector queue's preamble is longer.
            warm_mv = const_pool.tile([P, 512], dt.float16)
            nc.gpsimd.memset(warm_mv[:], 0.0)
            warm_ps = acc_pool.tile([P, 512], dt.float32, tag="acc", name="warm")
            for _ in range(N_WARM):
                nc.tensor.matmul(
                    warm_ps[:], warm_mv[:, 0:P], warm_mv[:], start=True, stop=True
                )

            # ---- sync-ring DMAs in consumption order (FIFO within a ring)
            wrep = const_pool.tile([P, D], dt.float32)
            k_sb = kv_pool.tile([P, NT * D], dt.float32, tag="ksb")
            v_sb = kv_pool.tile([P, NT * VP], dt.float16, tag="vsb")
            v3 = v_sb[:].rearrange("p (s n) -> p s n", n=VP)
            mask16 = kv_pool.tile([P, NT * L], dt.float16, tag="m16")
            m16v = mask16[:].rearrange("p (s i) -> p s i", s=NT)

            nc.sync.dma_start(wrep[:], wrep_d[:])
            mu8 = {}

            def load_pair(pi):
                a, bb = U8_PAIRS[pi]
                t8 = mu8_pool.tile([P, 2 * L], dt.uint8, tag="mu8", name=f"mu8_{a}")
                nc.sync.dma_start(t8[:], maskt_d[:, a * L : (bb + 1) * L])
                mu8[a] = t8
                mu8[bb] = t8

            # chunk 0 of k/v split per-strip for the fastest sk->e start
            for s in range(4):
                nc.sync.dma_start(k_sb[:, s * D : (s + 1) * D], key_d[:, s * D : (s + 1) * D])
            nc.sync.dma_start(v_sb[:, 0 : 4 * VP], value_d[:, 0 : 4 * VP])
            load_pair(0)
            for c in range(1, 4):
                ks = slice(c * 4 * D, (c + 1) * 4 * D)
                vs = slice(c * 4 * VP, (c + 1) * 4 * VP)
                nc.sync.dma_start(k_sb[:, ks], key_d[:, ks])
                nc.sync.dma_start(v_sb[:, vs], value_d[:, vs])
                load_pair(c)
            load_pair(4)

            # ---- SWDGE ring: cast strips u8 -> f16 directly into mask16
            for s in CAST_STRIPS:
                sl = slice(s * L, (s + 1) * L)
                nc.gpsimd.dma_start(mask16[:, sl], maskt_d[:, sl])

            # ---- prologue per chunk of 4 strips: sk = k.w ; e = exp(sk) ;
            # then scaled-mask conversions / ev rows for cast strips.
            sk = small_pool.tile([P, NT], dt.float32, tag="sk")
            e_sb = small_pool.tile([P, NT], dt.float32, tag="e")
            ev = kv_pool.tile([P, NT * VP], dt.float16, tag="ev")
            ev3 = ev[:].rearrange("p (s n) -> p s n", n=VP)

            def convert(s):
                if s in CAST_STRIPS:
                    # moving for this strip: ev row [e*v | e | 0]
                    nc.vector.tensor_scalar_mul(
                        ev3[:, s, 0:D], v3[:, s, 0:D], e_sb[:, s : s + 1]
                    )
                    nc.vector.tensor_copy(ev3[:, s : s + 1, D], e_sb[:, s : s + 1])
                elif s in DVE_STRIPS:
                    nc.vector.tensor_scalar_mul(
                        mask16[:, s * L : (s + 1) * L], mu8[s][:, 0:L], e_sb[:, s : s + 1]
                    )
                else:
                    nc.scalar.mul(
                        mask16[:, s * L : (s + 1) * L], mu8[s][:, L : 2 * L], e_sb[:, s : s + 1]
                    )

            for c in range(4):
                for s in range(4 * c, 4 * c + 4):
                    junk = junk_pool.tile([P, D], dt.float32, tag="junk")
                    nc.vector.scalar_tensor_tensor(
                        out=junk[:],
                        in0=k_sb[:, s * D : (s + 1) * D],
                        scalar=1.0,
                        in1=wrep[:],
                        op0=mybir.AluOpType.mult,
                        op1=mybir.AluOpType.mult,
                        accum_out=sk[:, s : s + 1],
                    )
                    if c == 0:
                        # singleton exp: unlock strip s without waiting the chunk
                        nc.scalar.activation(
                            e_sb[:, s : s + 1],
                            sk[:, s : s + 1],
                            mybir.ActivationFunctionType.Exp,
                        )
                        convert(s)
                if c > 0:
                    cs = slice(4 * c, 4 * c + 4)
                    nc.scalar.activation(
                        e_sb[:, cs], sk[:, cs], mybir.ActivationFunctionType.Exp
                    )
                    for s in range(4 * c, 4 * c + 4):
                        convert(s)

            # ---- two waves of 8 i-tiles; 16 accumulating matmuls each
            for w in range(2):
                accs = []
                for t in range(8 * w, 8 * w + 8):
                    accs.append(
                        acc_pool.tile([P, NE], dt.float32, tag="acc", name=f"acc{t}")
                    )
                for s in range(NT):
                    mov = ev3 if s in CAST_STRIPS else v3
                    for ti, t in enumerate(range(8 * w, 8 * w + 8)):
                        nc.tensor.matmul(
                            accs[ti][:],
                            m16v[:, s, t * P : (t + 1) * P],
                            mov[:, s, 0:NE],
                            start=(s == 0),
                            stop=(s == NT - 1),
                        )
                outb = out_pool.tile([P, 8 * D], dt.float16, tag="outb", name=f"outb{w}")
                for ti, t in enumerate(range(8 * w, 8 * w + 8)):
                    acc = accs[ti]
                    rec = rec_pool.tile([P, 1], dt.float32, tag="rec")
                    nc.vector.reciprocal(rec[:], acc[:, D : D + 1])
                    nc.scalar.mul(outb[:, ti * D : (ti + 1) * D], acc[:, 0:D], rec[:])
                nc.scalar.dma_start(out_d[:, w * 8 * D : (w + 1) * 8 * D], outb[:])

    nc.compile()
    return nc


def kernel(query, key, value, mask, w_align):
    global LAST_RESULTS
    key = np.asarray(key, dtype=np.float32)
    value = np.asarray(value, dtype=np.float32)
    mask = np.asarray(mask)
    w_align = np.asarray(w_align, dtype=np.float32)
    wrep = np.ascontiguousarray(np.tile(w_align[None, :], (P, 1)))

    nc = _build_nc()
    in_maps = []
    for b in range(B):
        # maskt[p, s, t*128+c] = mask[b][i=128t+c, j=128s+p]
        mt = (
            mask[b]
            .astype(np.uint8)
            .reshape(NT, P, NT, P)  # [t, c, s, p]
            .transpose(3, 2, 0, 1)  # [p, s, t, c]
            .reshape(P, NT * L)
        )
        kb = np.ascontiguousarray(
            key[b].reshape(NT, P, D).transpose(1, 0, 2).reshape(P, NT * D)
        )
        # value with ones-column (denominator) and zero pad, pitch 258
        vb = np.zeros((P, NT, VP), dtype=np.float16)
        vb[:, :, 0:D] = value[b].reshape(NT, P, D).transpose(1, 0, 2)
        vb[:, :, D] = 1.0
        in_maps.append(
            {
                "maskt": np.ascontiguousarray(mt),
                "key": kb,
                "value": vb.reshape(P, NT * VP),
                "wrep": wrep,
            }
        )
    try:
        res = run_bass_kernel_spmd(nc, in_maps, core_ids=list(range(B)))
    except Exception:
        # e.g. trace requested but profiling unavailable -- retry untraced
        os.environ["BASS_NEVER_TRACE"] = "1"
        res = run_bass_kernel_spmd(nc, in_maps, core_ids=list(range(B)))
    LAST_RESULTS = res
    out = np.empty((B, L, D), dtype=np.float32)
    for b in range(B):
        ob = res.results[b]["out"].astype(np.float32)  # [p, t*D]
        out[b] = ob.reshape(P, NT, D).transpose(1, 0, 2).reshape(L, D)
    return out
